# revision 14
# baseline (speedup 1.0000x reference)
"""AttnBlock Trainium2 Bass kernel.

Data-parallel over batch across 8 NeuronCores (4 batch elements each, full
weights everywhere). Everything on-chip is kept feature-major ([feat, token])
so no transposes are ever needed:

  x[b]               -> X   [C=256, N=1024]   (natural layout of the input)
  QK proj            -> Qst/Kst [128, 1024]   two heads stacked (64 rows each)
  V proj             -> V   [N, 4*65]         [j, d] layout + ones column
  scores (transposed)-> ST  [j, i] = K^T Q    row-packed: 2 heads concurrently
  P = exp(scale*ST)  -> ACT engine, scale folded into ACT's free affine
  ctx = [V|1]^T P    -> [65, 1024] psum: row 64 accumulates Z = sum_j P[j,i]
  normalize          -> PE broadcast of Z + fast reciprocal + DVE multiply
  out proj           -> OUT^T [C, N] + bias + residual fused on DVE
"""

import numpy as np
import ml_dtypes

N_HEADS = 4
D_K = 64
SCALE = D_K ** (-0.5)
B, C, H, W = 32, 256, 32, 32
N = H * W           # 1024 tokens
NCORES = 8
BPC = B // NCORES   # 4 batch elements per core

_CACHE = {}


def _build():
    import concourse.bacc as bacc
    import concourse.mybir as mybir
    from concourse.tile import TileContext

    dt = mybir.dt
    f32 = dt.float32
    bf16 = dt.bfloat16
    EXP = mybir.ActivationFunctionType.Exp
    ADD = mybir.AluOpType.add
    MULT = mybir.AluOpType.mult

    nc = bacc.Bacc()
    x = nc.dram_tensor("x", [BPC, C, N], f32, kind="ExternalInput")
    xbf = nc.dram_tensor("xbf", [BPC, C, N], bf16, kind="ExternalInput")
    wqk = nc.dram_tensor("wqk", [C, 512], bf16, kind="ExternalInput")
    bqk = nc.dram_tensor("bqk", [128, 4], f32, kind="ExternalInput")
    wv = nc.dram_tensor("wv", [C, 260], bf16, kind="ExternalInput")
    wvb = nc.dram_tensor("wvb", [128, 260], f32, kind="ExternalInput")
    wo = nc.dram_tensor("wo", [C, C], bf16, kind="ExternalInput")
    ob = nc.dram_tensor("ob", [128, 2], f32, kind="ExternalInput")
    out = nc.dram_tensor("out", [BPC, C, N], f32, kind="ExternalOutput")

    with TileContext(nc) as tc:
        with (
            tc.tile_pool(name="consts", bufs=1) as consts,
            tc.tile_pool(name="xp", bufs=2) as xp,
            tc.tile_pool(name="qkp", bufs=2) as qkp,
            tc.tile_pool(name="vp", bufs=2) as vp,
            tc.tile_pool(name="pp", bufs=3) as pp,
            tc.tile_pool(name="miscp", bufs=2) as miscp,
            tc.tile_pool(name="outp", bufs=4) as outp,
            tc.tile_pool(name="psum", bufs=2, space="PSUM") as psum,
        ):
            # ---- load constants once (already bf16 host-side) ----
            wqk_sb = [consts.tile([128, 512], bf16, name=f"wqk{cc}") for cc in range(2)]
            wv_sb = [consts.tile([128, 260], bf16, name=f"wv{cc}") for cc in range(2)]
            wo_sb = [consts.tile([128, 256], bf16, name=f"wo{cc}") for cc in range(2)]
            bqk_sb = consts.tile([128, 4], f32, name="bqk_sb")
            wvb_sb = consts.tile([128, 260], f32, name="wvb_sb")
            ob_sb = consts.tile([128, 2], f32, name="ob_sb")
            for cc in range(2):
                nc.sync.dma_start(out=wqk_sb[cc][:], in_=wqk[cc * 128:(cc + 1) * 128, :])
                nc.sync.dma_start(out=wv_sb[cc][:], in_=wv[cc * 128:(cc + 1) * 128, :])
                nc.sync.dma_start(out=wo_sb[cc][:], in_=wo[cc * 128:(cc + 1) * 128, :])
            nc.sync.dma_start(out=bqk_sb[:], in_=bqk[:])
            nc.sync.dma_start(out=wvb_sb[:], in_=wvb[:])
            nc.sync.dma_start(out=ob_sb[:], in_=ob[:])
            warmup = consts.tile([1, 4], f32, name="warmup")
            nc.scalar.activation(warmup[:], bqk_sb[0:1, 0:4], EXP)

            for b in range(BPC):
                # ---- load X ----
                xc = [xp.tile([128, N], f32, name=f"xc{cc}") for cc in range(2)]
                xcr = [xp.tile([128, N], bf16, name=f"xcr{cc}") for cc in range(2)]
                for cc in range(2):
                    nc.sync.dma_start(out=xc[cc][:], in_=x[b, cc * 128:(cc + 1) * 128, :])
                    nc.sync.dma_start(out=xcr[cc][:], in_=xbf[b, cc * 128:(cc + 1) * 128, :])

                # ---- QK projections: per pack p, Qst/Kst [128, 1024] ----
                qk_sb = []  # [p][0]=Qst, [p][1]=Kst
                for p in range(2):
                    pair = []
                    for qk in range(2):
                        qkps = psum.tile([128, N], f32, name="bigps", tag="big")
                        col0 = p * 256 + qk * 128
                        for fc in range(2):
                            fs = slice(fc * 512, (fc + 1) * 512)
                            for cc in range(2):
                                nc.tensor.matmul(
                                    qkps[:, fs],
                                    wqk_sb[cc][:, col0:col0 + 128],
                                    xcr[cc][:, fs],
                                    start=(cc == 0), stop=(cc == 1),
                                )
                        t = qkp.tile([128, N], bf16, name=f"qk{p}{qk}")
                        nc.vector.tensor_scalar(
                            t[:], qkps[:], bqk_sb[:, 2 * p + qk:2 * p + qk + 1],
                            None, ADD,
                        )
                        pair.append(t)
                    qk_sb.append(pair)

                # ---- V projection: [N, 260] as 8 j-tiles ----
                v_sb = vp.tile([128, 8, 260], bf16, name="v_sb")
                for jt in range(8):
                    vps = psum.tile([128, 260], f32, name="vps", tag="big")
                    js = slice(jt * 128, (jt + 1) * 128)
                    for cc in range(2):
                        nc.tensor.matmul(
                            vps[:], xcr[cc][:, js], wv_sb[cc][:],
                            start=(cc == 0), stop=(cc == 1),
                        )
                    nc.vector.scalar_tensor_tensor(
                        v_sb[:, jt, :], vps[:], 1.0, wvb_sb[:],
                        MULT, ADD,
                    )

                # ---- attention helpers (emitted via closures for pipelining) ----
                def emit_pack(p, qk_sb=qk_sb, v_sb=v_sb, b=b):
                    qst, kst = qk_sb[p][0], qk_sb[p][1]
                    ctxps = [
                        psum.tile([65, N], f32, name=f"ctx{hl}", tag=f"ctx{hl}", bufs=1)
                        for hl in range(2)
                    ]
                    pts = []
                    for jc in range(8):
                        js = slice(jc * 128, (jc + 1) * 128)
                        stps = [
                            psum.tile([128, N], f32, name=f"st{hl}", tag="big")
                            for hl in range(2)
                        ]
                        for ic in range(2):
                            isl = slice(ic * 512, (ic + 1) * 512)
                            for hl in range(2):
                                hs = slice(hl * 64, (hl + 1) * 64)
                                nc.tensor.matmul(
                                    stps[hl][:, isl],
                                    kst[hs, js],
                                    qst[hs, isl],
                                    start=True, stop=True,
                                )
                        pt = [pp.tile([128, N], bf16, name=f"p{hl}") for hl in range(2)]
                        for hl in range(2):
                            nc.scalar.activation(pt[hl][:], stps[hl][:], EXP, scale=SCALE)
                        for hl in range(2):
                            h = 2 * p + hl
                            for ic in range(2):
                                isl = slice(ic * 512, (ic + 1) * 512)
                                nc.tensor.matmul(
                                    ctxps[hl][:, isl],
                                    v_sb[:, jc, h * 65:(h + 1) * 65],
                                    pt[hl][:, isl],
                                    start=(jc == 0), stop=(jc == 7),
                                )
                    # Z rows -> sbuf, reciprocal, partition-broadcast normalize
                    z_sb = miscp.tile([1, 2 * N], f32, name="z_sb")
                    for hl in range(2):
                        nc.vector.tensor_copy(
                            z_sb[0:1, hl * N:(hl + 1) * N], ctxps[hl][64:65, :]
                        )
                    zb = miscp.tile([64, 2 * N], f32, name="zb")
                    nc.gpsimd.partition_broadcast(zb[:], z_sb[0:1, :])
                    rzb = miscp.tile([64, 2 * N], f32, name="rzb")
                    nc.vector.reciprocal_approx_fast(rzb[:], zb[:])
                    cn = miscp.tile([128, N], bf16, name=f"ctxn{p}")
                    for hl in range(2):
                        nc.vector.tensor_tensor(
                            cn[hl * 64:(hl + 1) * 64, :],
                            ctxps[hl][0:64, :],
                            rzb[0:64, hl * N:(hl + 1) * N],
                            MULT,
                        )
                    return cn

                ctxn = [emit_pack(0), emit_pack(1)]

                # ---- output projection + bias + residual ----
                for co in range(2):
                    ops = psum.tile([128, N], f32, name="ops", tag=f"ctx{co}", bufs=1)
                    for ic in range(2):
                        isl = slice(ic * 512, (ic + 1) * 512)
                        for kc in range(2):
                            nc.tensor.matmul(
                                ops[:, isl],
                                wo_sb[kc][:, co * 128:(co + 1) * 128],
                                ctxn[kc][:, isl],
                                start=(kc == 0), stop=(kc == 1),
                            )
                    osb = outp.tile([128, N], f32, name="osb")
                    nc.vector.scalar_tensor_tensor(
                        osb[:], ops[:], ob_sb[:, co:co + 1], xc[co][:], ADD, ADD
                    )
                    nc.sync.dma_start(
                        out=out[b, co * 128:(co + 1) * 128, :], in_=osb[:]
                    )

    nc.compile()
    return nc


def _prep_weights(proj_w, proj_b, out_w, out_b):
    qk_cols = []
    for p in range(2):
        for qk in range(2):
            for hl in range(2):
                h = 2 * p + hl
                base = h * 192 + qk * 64
                qk_cols.extend(range(base, base + 64))
    wqk = np.ascontiguousarray(proj_w[qk_cols, :].T).astype(ml_dtypes.bfloat16)
    bqk = np.ascontiguousarray(
        proj_b[qk_cols].reshape(4, 128).T                     # [128, 4]
    )

    wv = np.zeros((C, 260), dtype=np.float32)
    wvb1 = np.zeros((1, 260), dtype=np.float32)
    for h in range(N_HEADS):
        rows = range(h * 192 + 128, h * 192 + 192)
        wv[:, h * 65:h * 65 + 64] = proj_w[rows, :].T
        wvb1[0, h * 65:h * 65 + 64] = proj_b[rows]
        wvb1[0, h * 65 + 64] = 1.0
    wvb = np.ascontiguousarray(np.repeat(wvb1, 128, axis=0))  # [128, 260]
    wv = wv.astype(ml_dtypes.bfloat16)

    wo = np.ascontiguousarray(out_w.T).astype(ml_dtypes.bfloat16)
    ob = np.ascontiguousarray(out_b.reshape(2, 128).T)        # [128, 2]
    return dict(wqk=wqk, bqk=bqk, wv=wv, wvb=wvb, wo=wo, ob=ob)


def kernel(x, proj_w, proj_b, out_w, out_b, _trace=False):
    from concourse.bass_utils import run_bass_kernel_spmd

    x = np.asarray(x, dtype=np.float32)
    proj_w = np.asarray(proj_w, dtype=np.float32)
    proj_b = np.asarray(proj_b, dtype=np.float32)
    out_w = np.asarray(out_w, dtype=np.float32)
    out_b = np.asarray(out_b, dtype=np.float32)

    if "nc" not in _CACHE:
        _CACHE["nc"] = _build()
    nc = _CACHE["nc"]

    w = _prep_weights(proj_w, proj_b, out_w, out_b)
    xs = np.ascontiguousarray(x.reshape(B, C, N))
    xsbf = xs.astype(ml_dtypes.bfloat16)
    in_maps = [
        dict(w, x=np.ascontiguousarray(xs[i * BPC:(i + 1) * BPC]),
             xbf=np.ascontiguousarray(xsbf[i * BPC:(i + 1) * BPC]))
        for i in range(NCORES)
    ]
    res = run_bass_kernel_spmd(nc, in_maps, core_ids=list(range(NCORES)), trace=_trace)
    out = np.concatenate([r["out"] for r in res.results], axis=0)
    out = out.reshape(B, C, H, W)
    if _trace:
        _CACHE["last_result"] = res
    return out


# revision 15
# speedup vs baseline: 1.2352x; 1.2352x over previous
"""AttnBlock Trainium2 Bass kernel.

Data-parallel over batch across 8 NeuronCores (4 batch elements each, full
weights everywhere). Everything on-chip is kept feature-major ([feat, token])
so no transposes are ever needed:

  x[b]               -> X   [C=256, N=1024]   (natural layout of the input)
  QK proj            -> Qst/Kst [128, 1024]   two heads stacked (64 rows each)
  V proj             -> V   [N, 4*65]         [j, d] layout + ones column
  scores (transposed)-> ST  [j, i] = K^T Q    row-packed: 2 heads concurrently
  P = exp(scale*ST)  -> ACT engine, scale folded into ACT's free affine
  ctx = [V|1]^T P    -> [65, 1024] psum: row 64 accumulates Z = sum_j P[j,i]
  normalize          -> PE broadcast of Z + fast reciprocal + DVE multiply
  out proj           -> OUT^T [C, N] + bias + residual fused on DVE
"""

import numpy as np
import ml_dtypes

N_HEADS = 4
D_K = 64
SCALE = D_K ** (-0.5)
B, C, H, W = 32, 256, 32, 32
N = H * W           # 1024 tokens
NCORES = 8
BPC = B // NCORES   # 4 batch elements per core

_CACHE = {}


def _build():
    import concourse.bacc as bacc
    import concourse.mybir as mybir
    from concourse.tile import TileContext

    dt = mybir.dt
    f32 = dt.float32
    bf16 = dt.bfloat16
    EXP = mybir.ActivationFunctionType.Exp
    ADD = mybir.AluOpType.add
    MULT = mybir.AluOpType.mult

    nc = bacc.Bacc()
    x = nc.dram_tensor("x", [BPC, C, N], f32, kind="ExternalInput")
    xbf = nc.dram_tensor("xbf", [BPC, C, N], bf16, kind="ExternalInput")
    wqk = nc.dram_tensor("wqk", [C, 512], bf16, kind="ExternalInput")
    bqk = nc.dram_tensor("bqk", [128, 4], f32, kind="ExternalInput")
    wv = nc.dram_tensor("wv", [C, 260], bf16, kind="ExternalInput")
    wvb = nc.dram_tensor("wvb", [128, 260], f32, kind="ExternalInput")
    wo = nc.dram_tensor("wo", [C, C], bf16, kind="ExternalInput")
    ob = nc.dram_tensor("ob", [128, 2], f32, kind="ExternalInput")
    out = nc.dram_tensor("out", [BPC, C, N], f32, kind="ExternalOutput")

    with TileContext(nc) as tc:
        with (
            tc.tile_pool(name="consts", bufs=1) as consts,
            tc.tile_pool(name="xp", bufs=2) as xp,
            tc.tile_pool(name="qkp", bufs=2) as qkp,
            tc.tile_pool(name="vp", bufs=2) as vp,
            tc.tile_pool(name="pp", bufs=3) as pp,
            tc.tile_pool(name="miscp", bufs=2) as miscp,
            tc.tile_pool(name="outp", bufs=4) as outp,
            tc.tile_pool(name="psum", bufs=2, space="PSUM") as psum,
        ):
            # ---- load constants once (already bf16 host-side) ----
            wqk_sb = [consts.tile([128, 512], bf16, name=f"wqk{cc}") for cc in range(2)]
            wv_sb = [consts.tile([128, 260], bf16, name=f"wv{cc}") for cc in range(2)]
            wo_sb = [consts.tile([128, 256], bf16, name=f"wo{cc}") for cc in range(2)]
            bqk_sb = consts.tile([128, 4], f32, name="bqk_sb")
            wvb_sb = consts.tile([128, 260], f32, name="wvb_sb")
            ob_sb = consts.tile([128, 2], f32, name="ob_sb")
            for cc in range(2):
                nc.sync.dma_start(out=wqk_sb[cc][:], in_=wqk[cc * 128:(cc + 1) * 128, :])
                nc.sync.dma_start(out=wv_sb[cc][:], in_=wv[cc * 128:(cc + 1) * 128, :])
                nc.sync.dma_start(out=wo_sb[cc][:], in_=wo[cc * 128:(cc + 1) * 128, :])
            nc.sync.dma_start(out=bqk_sb[:], in_=bqk[:])
            nc.sync.dma_start(out=wvb_sb[:], in_=wvb[:])
            nc.sync.dma_start(out=ob_sb[:], in_=ob[:])
            warmup = consts.tile([1, 4], f32, name="warmup")
            nc.scalar.activation(warmup[:], bqk_sb[0:1, 0:4], EXP)

            # ================= phase 1: QKV for ALL batch elements =========
            xcs, qks, vss = [], [], []
            for b in range(BPC):
                xc = [xp.tile([128, N], f32, name=f"xc{cc}", bufs=4) for cc in range(2)]
                xcr = [xp.tile([128, N], bf16, name=f"xcr{cc}", bufs=4) for cc in range(2)]
                for cc in range(2):
                    nc.sync.dma_start(out=xc[cc][:], in_=x[b, cc * 128:(cc + 1) * 128, :])
                    nc.sync.dma_start(out=xcr[cc][:], in_=xbf[b, cc * 128:(cc + 1) * 128, :])
                xcs.append(xc)

                qk_sb = []  # [p][0]=Qst, [p][1]=Kst
                for p in range(2):
                    pair = []
                    for qk in range(2):
                        qkps = psum.tile([128, N], f32, name="bigps", tag="big")
                        col0 = p * 256 + qk * 128
                        for fc in range(2):
                            fs = slice(fc * 512, (fc + 1) * 512)
                            for cc in range(2):
                                nc.tensor.matmul(
                                    qkps[:, fs],
                                    wqk_sb[cc][:, col0:col0 + 128],
                                    xcr[cc][:, fs],
                                    start=(cc == 0), stop=(cc == 1),
                                )
                        t = qkp.tile([128, N], bf16, name=f"qk{p}{qk}", bufs=4)
                        nc.vector.tensor_scalar(
                            t[:], qkps[:], bqk_sb[:, 2 * p + qk:2 * p + qk + 1],
                            None, ADD,
                        )
                        pair.append(t)
                    qk_sb.append(pair)
                qks.append(qk_sb)

                v_sb = vp.tile([128, 8, 260], bf16, name="v_sb", bufs=4)
                for jt in range(8):
                    vps = psum.tile([128, 260], f32, name="vps", tag="big")
                    js = slice(jt * 128, (jt + 1) * 128)
                    for cc in range(2):
                        nc.tensor.matmul(
                            vps[:], xcr[cc][:, js], wv_sb[cc][:],
                            start=(cc == 0), stop=(cc == 1),
                        )
                    nc.vector.scalar_tensor_tensor(
                        v_sb[:, jt, :], vps[:], 1.0, wvb_sb[:],
                        MULT, ADD,
                    )
                vss.append(v_sb)

            # ============ phase 2: attention stream + interleaved outproj ==
            def emit_pack(b, p):
                qst, kst = qks[b][p][0], qks[b][p][1]
                v_sb = vss[b]
                ctxps = [
                    psum.tile([65, N], f32, name=f"ctx{hl}", tag=f"ctx{hl}", bufs=1)
                    for hl in range(2)
                ]
                for jc in range(8):
                    js = slice(jc * 128, (jc + 1) * 128)
                    stps = [
                        psum.tile([128, N], f32, name=f"st{hl}", tag="big")
                        for hl in range(2)
                    ]
                    for ic in range(2):
                        isl = slice(ic * 512, (ic + 1) * 512)
                        for hl in range(2):
                            hs = slice(hl * 64, (hl + 1) * 64)
                            nc.tensor.matmul(
                                stps[hl][:, isl],
                                kst[hs, js],
                                qst[hs, isl],
                                start=True, stop=True,
                            )
                    pt = [pp.tile([128, N], bf16, name=f"p{hl}") for hl in range(2)]
                    for hl in range(2):
                        nc.scalar.activation(pt[hl][:], stps[hl][:], EXP, scale=SCALE)
                    for hl in range(2):
                        h = 2 * p + hl
                        for ic in range(2):
                            isl = slice(ic * 512, (ic + 1) * 512)
                            nc.tensor.matmul(
                                ctxps[hl][:, isl],
                                v_sb[:, jc, h * 65:(h + 1) * 65],
                                pt[hl][:, isl],
                                start=(jc == 0), stop=(jc == 7),
                            )
                # per-head: Z row -> sbuf, gpsimd broadcast, recip, normalize
                cn = miscp.tile([128, N], bf16, name=f"ctxn{p}", bufs=2)
                for hl in range(2):
                    z_sb = miscp.tile([1, N], f32, name="z_sb", bufs=4)
                    nc.vector.tensor_copy(z_sb[:], ctxps[hl][64:65, :])
                    zb = miscp.tile([64, N], f32, name="zb", bufs=4)
                    nc.gpsimd.partition_broadcast(zb[:], z_sb[0:1, :])
                    rzb = miscp.tile([64, N], f32, name="rzb", bufs=4)
                    nc.vector.reciprocal_approx_fast(rzb[:], zb[:])
                    nc.vector.tensor_tensor(
                        cn[hl * 64:(hl + 1) * 64, :],
                        ctxps[hl][0:64, :],
                        rzb[:],
                        MULT,
                    )
                return cn

            def emit_outproj(b, ctxn):
                for co in range(2):
                    ops = psum.tile([128, N], f32, name="ops", tag=f"ctx{co}", bufs=1)
                    for ic in range(2):
                        isl = slice(ic * 512, (ic + 1) * 512)
                        for kc in range(2):
                            nc.tensor.matmul(
                                ops[:, isl],
                                wo_sb[kc][:, co * 128:(co + 1) * 128],
                                ctxn[kc][:, isl],
                                start=(kc == 0), stop=(kc == 1),
                            )
                    osb = outp.tile([128, N], f32, name="osb")
                    nc.vector.scalar_tensor_tensor(
                        osb[:], ops[:], ob_sb[:, co:co + 1], xcs[b][co][:], ADD, ADD
                    )
                    nc.sync.dma_start(
                        out=out[b, co * 128:(co + 1) * 128, :], in_=osb[:]
                    )

            prev = None
            for b in range(BPC):
                cn0 = emit_pack(b, 0)
                if prev is not None:
                    emit_outproj(prev[0], prev[1])
                    prev = None
                cn1 = emit_pack(b, 1)
                prev = (b, [cn0, cn1])
            emit_outproj(prev[0], prev[1])

    nc.compile()
    return nc


def _prep_weights(proj_w, proj_b, out_w, out_b):
    qk_cols = []
    for p in range(2):
        for qk in range(2):
            for hl in range(2):
                h = 2 * p + hl
                base = h * 192 + qk * 64
                qk_cols.extend(range(base, base + 64))
    wqk = np.ascontiguousarray(proj_w[qk_cols, :].T).astype(ml_dtypes.bfloat16)
    bqk = np.ascontiguousarray(
        proj_b[qk_cols].reshape(4, 128).T                     # [128, 4]
    )

    wv = np.zeros((C, 260), dtype=np.float32)
    wvb1 = np.zeros((1, 260), dtype=np.float32)
    for h in range(N_HEADS):
        rows = range(h * 192 + 128, h * 192 + 192)
        wv[:, h * 65:h * 65 + 64] = proj_w[rows, :].T
        wvb1[0, h * 65:h * 65 + 64] = proj_b[rows]
        wvb1[0, h * 65 + 64] = 1.0
    wvb = np.ascontiguousarray(np.repeat(wvb1, 128, axis=0))  # [128, 260]
    wv = wv.astype(ml_dtypes.bfloat16)

    wo = np.ascontiguousarray(out_w.T).astype(ml_dtypes.bfloat16)
    ob = np.ascontiguousarray(out_b.reshape(2, 128).T)        # [128, 2]
    return dict(wqk=wqk, bqk=bqk, wv=wv, wvb=wvb, wo=wo, ob=ob)


def kernel(x, proj_w, proj_b, out_w, out_b, _trace=False):
    from concourse.bass_utils import run_bass_kernel_spmd

    x = np.asarray(x, dtype=np.float32)
    proj_w = np.asarray(proj_w, dtype=np.float32)
    proj_b = np.asarray(proj_b, dtype=np.float32)
    out_w = np.asarray(out_w, dtype=np.float32)
    out_b = np.asarray(out_b, dtype=np.float32)

    if "nc" not in _CACHE:
        _CACHE["nc"] = _build()
    nc = _CACHE["nc"]

    w = _prep_weights(proj_w, proj_b, out_w, out_b)
    xs = np.ascontiguousarray(x.reshape(B, C, N))
    xsbf = xs.astype(ml_dtypes.bfloat16)
    in_maps = [
        dict(w, x=np.ascontiguousarray(xs[i * BPC:(i + 1) * BPC]),
             xbf=np.ascontiguousarray(xsbf[i * BPC:(i + 1) * BPC]))
        for i in range(NCORES)
    ]
    res = run_bass_kernel_spmd(nc, in_maps, core_ids=list(range(NCORES)), trace=_trace)
    out = np.concatenate([r["out"] for r in res.results], axis=0)
    out = out.reshape(B, C, H, W)
    if _trace:
        _CACHE["last_result"] = res
    return out


# revision 17
# speedup vs baseline: 1.2427x; 1.0060x over previous
"""AttnBlock Trainium2 Bass kernel.

Data-parallel over batch across 8 NeuronCores (4 batch elements each, full
weights everywhere). Everything on-chip is kept feature-major ([feat, token])
so no transposes are ever needed:

  x[b]               -> X   [C=256, N=1024]   (natural layout of the input)
  QK proj            -> Qst/Kst [128, 1024]   two heads stacked (64 rows each)
  V proj             -> V   [N, 4*65]         [j, d] layout + ones column
  scores (transposed)-> ST  [j, i] = K^T Q    row-packed: 2 heads concurrently
  P = exp(scale*ST)  -> ACT engine, scale folded into ACT's free affine
  ctx = [V|1]^T P    -> [65, 1024] psum: row 64 accumulates Z = sum_j P[j,i]
  normalize          -> PE broadcast of Z + fast reciprocal + DVE multiply
  out proj           -> OUT^T [C, N] + bias + residual fused on DVE
"""

import numpy as np
import ml_dtypes

N_HEADS = 4
D_K = 64
SCALE = D_K ** (-0.5)
B, C, H, W = 32, 256, 32, 32
N = H * W           # 1024 tokens
NCORES = 8
BPC = B // NCORES   # 4 batch elements per core

_CACHE = {}


def _build():
    import concourse.bacc as bacc
    import concourse.mybir as mybir
    from concourse.tile import TileContext

    dt = mybir.dt
    f32 = dt.float32
    bf16 = dt.bfloat16
    EXP = mybir.ActivationFunctionType.Exp
    ADD = mybir.AluOpType.add
    MULT = mybir.AluOpType.mult

    nc = bacc.Bacc()
    x = nc.dram_tensor("x", [BPC, C, N], f32, kind="ExternalInput")
    xbf = nc.dram_tensor("xbf", [BPC, C, N], bf16, kind="ExternalInput")
    wqk = nc.dram_tensor("wqk", [C, 512], bf16, kind="ExternalInput")
    bqk = nc.dram_tensor("bqk", [128, 4], f32, kind="ExternalInput")
    wv = nc.dram_tensor("wv", [C, 260], bf16, kind="ExternalInput")
    wvb = nc.dram_tensor("wvb", [128, 260], f32, kind="ExternalInput")
    wo = nc.dram_tensor("wo", [C, C], bf16, kind="ExternalInput")
    ob = nc.dram_tensor("ob", [128, 2], f32, kind="ExternalInput")
    out = nc.dram_tensor("out", [BPC, C, N], f32, kind="ExternalOutput")

    with TileContext(nc) as tc:
        with (
            tc.tile_pool(name="consts", bufs=1) as consts,
            tc.tile_pool(name="xp", bufs=2) as xp,
            tc.tile_pool(name="qkp", bufs=2) as qkp,
            tc.tile_pool(name="vp", bufs=2) as vp,
            tc.tile_pool(name="pp", bufs=3) as pp,
            tc.tile_pool(name="miscp", bufs=2) as miscp,
            tc.tile_pool(name="outp", bufs=4) as outp,
            tc.tile_pool(name="psum", bufs=2, space="PSUM") as psum,
        ):
            # ---- load constants once (already bf16 host-side) ----
            wqk_sb = [consts.tile([128, 512], bf16, name=f"wqk{cc}") for cc in range(2)]
            wv_sb = [consts.tile([128, 260], bf16, name=f"wv{cc}") for cc in range(2)]
            wo_sb = [consts.tile([128, 256], bf16, name=f"wo{cc}") for cc in range(2)]
            bqk_sb = consts.tile([128, 4], f32, name="bqk_sb")
            wvb_sb = consts.tile([128, 260], f32, name="wvb_sb")
            ob_sb = consts.tile([128, 2], f32, name="ob_sb")
            for cc in range(2):
                nc.sync.dma_start(out=wqk_sb[cc][:], in_=wqk[cc * 128:(cc + 1) * 128, :])
                nc.sync.dma_start(out=wv_sb[cc][:], in_=wv[cc * 128:(cc + 1) * 128, :])
                nc.sync.dma_start(out=wo_sb[cc][:], in_=wo[cc * 128:(cc + 1) * 128, :])
            nc.sync.dma_start(out=bqk_sb[:], in_=bqk[:])
            nc.sync.dma_start(out=wvb_sb[:], in_=wvb[:])
            nc.sync.dma_start(out=ob_sb[:], in_=ob[:])
            warmup = consts.tile([1, 4], f32, name="warmup")
            nc.scalar.activation(warmup[:], bqk_sb[0:1, 0:4], EXP)

            # ================= phase 1: QKV for ALL batch elements =========
            xcs, qks, vss = [], [], []
            for b in range(BPC):
                xc = [xp.tile([128, N], f32, name=f"xc{cc}", bufs=4) for cc in range(2)]
                xcr = [xp.tile([128, N], bf16, name=f"xcr{cc}", bufs=4) for cc in range(2)]
                for cc in range(2):
                    nc.sync.dma_start(out=xc[cc][:], in_=x[b, cc * 128:(cc + 1) * 128, :])
                    nc.sync.dma_start(out=xcr[cc][:], in_=xbf[b, cc * 128:(cc + 1) * 128, :])
                xcs.append(xc)

                qk_sb = []  # [p][0]=Qst, [p][1]=Kst
                for p in range(2):
                    pair = []
                    for qk in range(2):
                        qkps = psum.tile([128, N], f32, name="bigps", tag="big")
                        col0 = p * 256 + qk * 128
                        for fc in range(2):
                            fs = slice(fc * 512, (fc + 1) * 512)
                            for cc in range(2):
                                nc.tensor.matmul(
                                    qkps[:, fs],
                                    wqk_sb[cc][:, col0:col0 + 128],
                                    xcr[cc][:, fs],
                                    start=(cc == 0), stop=(cc == 1),
                                )
                        t = qkp.tile([128, N], bf16, name=f"qk{p}{qk}", bufs=4)
                        nc.vector.tensor_scalar(
                            t[:], qkps[:], bqk_sb[:, 2 * p + qk:2 * p + qk + 1],
                            None, ADD,
                        )
                        pair.append(t)
                    qk_sb.append(pair)
                qks.append(qk_sb)

                v_sb = vp.tile([128, 8, 260], bf16, name="v_sb", bufs=4)
                for jt in range(8):
                    vps = psum.tile([128, 260], f32, name="vps", tag="big")
                    js = slice(jt * 128, (jt + 1) * 128)
                    for cc in range(2):
                        nc.tensor.matmul(
                            vps[:], xcr[cc][:, js], wv_sb[cc][:],
                            start=(cc == 0), stop=(cc == 1),
                        )
                    nc.vector.scalar_tensor_tensor(
                        v_sb[:, jt, :], vps[:], 1.0, wvb_sb[:],
                        MULT, ADD,
                    )
                vss.append(v_sb)

            # ============ phase 2: attention stream + interleaved outproj ==
            def emit_pack(b, p):
                qst, kst = qks[b][p][0], qks[b][p][1]
                v_sb = vss[b]
                ctxps = [
                    psum.tile([65, N], f32, name=f"ctx{hl}", tag=f"ctx{hl}", bufs=1)
                    for hl in range(2)
                ]
                for jc in range(8):
                    js = slice(jc * 128, (jc + 1) * 128)
                    stps = [
                        psum.tile([128, N], f32, name=f"st{hl}", tag="big")
                        for hl in range(2)
                    ]
                    for ic in range(2):
                        isl = slice(ic * 512, (ic + 1) * 512)
                        for hl in range(2):
                            hs = slice(hl * 64, (hl + 1) * 64)
                            nc.tensor.matmul(
                                stps[hl][:, isl],
                                kst[hs, js],
                                qst[hs, isl],
                                start=True, stop=True,
                            )
                    pt = [pp.tile([128, N], bf16, name=f"p{hl}") for hl in range(2)]
                    for hl in range(2):
                        nc.scalar.activation(pt[hl][:], stps[hl][:], EXP, scale=SCALE)
                    for hl in range(2):
                        h = 2 * p + hl
                        for ic in range(2):
                            isl = slice(ic * 512, (ic + 1) * 512)
                            nc.tensor.matmul(
                                ctxps[hl][:, isl],
                                v_sb[:, jc, h * 65:(h + 1) * 65],
                                pt[hl][:, isl],
                                start=(jc == 0), stop=(jc == 7),
                            )
                # per-head: Z row -> sbuf, gpsimd broadcast, recip, normalize
                cn = miscp.tile([128, N], bf16, name=f"ctxn{p}", bufs=2)
                for hl in range(2):
                    z_sb = miscp.tile([1, N], f32, name="z_sb", bufs=4)
                    nc.vector.tensor_copy(z_sb[:], ctxps[hl][64:65, :])
                    zb = miscp.tile([64, N], f32, name="zb", bufs=4)
                    nc.gpsimd.partition_broadcast(zb[:], z_sb[0:1, :])
                    rzb = miscp.tile([64, N], f32, name="rzb", bufs=4)
                    nc.vector.reciprocal_approx_fast(rzb[:], zb[:])
                    nc.vector.tensor_tensor(
                        cn[hl * 64:(hl + 1) * 64, :],
                        ctxps[hl][0:64, :],
                        rzb[:],
                        MULT,
                    )
                return cn

            def emit_outproj(b, ctxn):
                for co in range(2):
                    ops = psum.tile([128, N], f32, name="ops", tag=f"ctx{co}", bufs=1)
                    for ic in range(2):
                        isl = slice(ic * 512, (ic + 1) * 512)
                        for kc in range(2):
                            nc.tensor.matmul(
                                ops[:, isl],
                                wo_sb[kc][:, co * 128:(co + 1) * 128],
                                ctxn[kc][:, isl],
                                start=(kc == 0), stop=(kc == 1),
                            )
                    osb = outp.tile([128, N], f32, name="osb")
                    nc.vector.scalar_tensor_tensor(
                        osb[:], ops[:], ob_sb[:, co:co + 1], xcs[b][co][:], ADD, ADD
                    )
                    nc.sync.dma_start(
                        out=out[b, co * 128:(co + 1) * 128, :], in_=osb[:]
                    )

            prev = None
            for b in range(BPC):
                cn0 = emit_pack(b, 0)
                if prev is not None:
                    emit_outproj(prev[0], prev[1])
                    prev = None
                cn1 = emit_pack(b, 1)
                prev = (b, [cn0, cn1])
            emit_outproj(prev[0], prev[1])

    nc.compile()
    return nc


def _prep_weights(proj_w, proj_b, out_w, out_b):
    qk_cols = []
    for p in range(2):
        for qk in range(2):
            for hl in range(2):
                h = 2 * p + hl
                base = h * 192 + qk * 64
                qk_cols.extend(range(base, base + 64))
    wqk = np.ascontiguousarray(proj_w[qk_cols, :].T).astype(ml_dtypes.bfloat16)
    bqk = np.ascontiguousarray(
        proj_b[qk_cols].reshape(4, 128).T                     # [128, 4]
    )

    wv = np.zeros((C, 260), dtype=np.float32)
    wvb1 = np.zeros((1, 260), dtype=np.float32)
    for h in range(N_HEADS):
        rows = range(h * 192 + 128, h * 192 + 192)
        wv[:, h * 65:h * 65 + 64] = proj_w[rows, :].T
        wvb1[0, h * 65:h * 65 + 64] = proj_b[rows]
        wvb1[0, h * 65 + 64] = 1.0
    wvb = np.ascontiguousarray(np.repeat(wvb1, 128, axis=0))  # [128, 260]
    wv = wv.astype(ml_dtypes.bfloat16)

    wo = np.ascontiguousarray(out_w.T).astype(ml_dtypes.bfloat16)
    ob = np.ascontiguousarray(out_b.reshape(2, 128).T)        # [128, 2]
    return dict(wqk=wqk, bqk=bqk, wv=wv, wvb=wvb, wo=wo, ob=ob)


def kernel(x, proj_w, proj_b, out_w, out_b, _trace=False):
    from concourse.bass_utils import run_bass_kernel_spmd

    x = np.asarray(x, dtype=np.float32)
    proj_w = np.asarray(proj_w, dtype=np.float32)
    proj_b = np.asarray(proj_b, dtype=np.float32)
    out_w = np.asarray(out_w, dtype=np.float32)
    out_b = np.asarray(out_b, dtype=np.float32)

    if "nc" not in _CACHE:
        _CACHE["nc"] = _build()
    nc = _CACHE["nc"]

    w = _prep_weights(proj_w, proj_b, out_w, out_b)
    xs = np.ascontiguousarray(x.reshape(B, C, N))
    xsbf = xs.astype(ml_dtypes.bfloat16)
    in_maps = [
        dict(w, x=np.ascontiguousarray(xs[i * BPC:(i + 1) * BPC]),
             xbf=np.ascontiguousarray(xsbf[i * BPC:(i + 1) * BPC]))
        for i in range(NCORES)
    ]
    res = run_bass_kernel_spmd(nc, in_maps, core_ids=list(range(NCORES)), trace=_trace)
    out = np.concatenate([r["out"] for r in res.results], axis=0)
    out = out.reshape(B, C, H, W)
    if _trace:
        _CACHE["last_result"] = res
    return out
